# revision 35
# baseline (speedup 1.0000x reference)
"""GQA (softermax) Trainium2 kernel, tensor-parallel over kv-head groups.

Problem: x[1,2048,4096], 32 q-heads / 8 kv-heads, head_dim=128, base-2
softmax (softermax), fp32 reference. Each of the 8 cores owns one kv-head
group (4 q-heads, 512 q dims, 128 kv dims) and computes a partial
o-projection [2048,4096]; the host sums the 8 partials and adds o_b.

v5 (pipelined, all-bf16; 458us -> ~421us). fp8/DoubleRow was evaluated
and rejected: softmax rows here are peaked (N_eff down to ~40), so fp8
quantization of P or x costs 2.2e-2..4.8e-2 max-rel error vs the 2e-2
gate. All matmuls stay bf16; the wins are scheduling:
 - k_b dropped entirely (adds a per-q constant to scores -> cancels in
   softmax); v_b folded into a host-side constant (sum_k P/Z = 1, so the
   V-bias contribution to out is v_bias_full @ o_w.T, independent of s).
 - PE warm-up: dummy matmuls during the initial DMA wait keep the HAM
   clock-gate warm so real matmuls start at 2.4 GHz.
 - Phase 1 is a group-burst interleaved stream (per e-group: K,V,Q0..Q3
   bursts accumulating into 6 live PSUM banks) so the weight/x DMA
   need-curve stays under the slow early DMA ramp and the first matmul
   needs only 1.25MB; urgent DMAs dispatch in parallel from
   scalar/gpsimd/sync queues in need order; bulk x-blocks are paced by
   data-probe reads on the gpsimd queue; PSUM->SBUF copies run inline
   after each stream's last burst, alternating ACT/DVE.
 - Phase-2 software pipeline: per (head h, chunk-pair kp) slot emits
   2 score MMs + exp + 2 PV MMs of head h-1 + one o-proj group (sl,ec)
   of the previous q-block, so the PE never waits on the ACT exp and
   the o-projection fills all exp-latency gaps. Z chunk-adds (DVE) are
   emitted per-slot; the Z broadcast matmul is deferred one slot.
 - Q(sb3) heads 0,1 are deferred from phase 1 into q-block-0's slots
   (which have no o-proj work and would otherwise be exp-paced).
 - o-proj output DMA'd per half-row-block (512KB); the final row block
   stores column-pairs on the scalar queue to drain the tail.

Per-core dataflow:
  proj:  xT[e,s] (DMA, host-chunked) ; K^T,V^T,Q^T = W^T.T @ xT
         V natural via 16 PE transposes of V^T
  attn:  S^T[k,q] = KT_chunk.T @ QT (pairs of chunks into one 2-bank
         PSUM tile); P^T = exp(S^T * ln2/sqrt(128)) [ACT, FD=1024]
         O^T[d,q] = sum_k Vn_chunk.T @ P^T_chunk   (PSUM accum)
         acc = bf16 chunk-add chain of P^T [DVE]; Z = ones128.T @ acc
         OTb = O^T * recip_approx(Z)               [DVE]
  oproj: out[s,e] = sum_h OTb_h_chunk.T @ owT_h    (partial; host sums)
"""

import math
from contextlib import ExitStack

import numpy as np
import ml_dtypes

import concourse.bass as bass
from concourse import bacc
import concourse.mybir as mybir
import concourse.tile as tile
from concourse.bass_utils import run_bass_kernel_spmd
from concourse.masks import make_identity

E = 4096          # embed dim
S = 2048          # sequence
D = 128           # head dim
NHL = 4           # q heads per core
DQ = NHL * D      # 512 q dims per core
DKV = 128         # kv dims per core (1 kv head)
NCORES = 8
NE = E // 128     # 32 embed chunks

SB = 512          # seq block for projection pass
NSB = S // SB
QS = 512          # q block in attention
NQS = S // QS
NKT = S // 128    # 16 k chunks
NOB = S // 128    # 16 output row blocks
NOE = E // 512    # 8 output col blocks

F32 = mybir.dt.float32
BF = mybir.dt.bfloat16
BF_NP = ml_dtypes.bfloat16
EXP_SCALE = math.log(2.0) / math.sqrt(D)

_CACHED_NC = None


def build_bass():
    nc = bacc.Bacc(None)

    # all inputs pre-chunked on host into SBUF tile layout [p, chunk, cols]
    xt_d = nc.declare_dram_parameter("xt", [NSB, 128, NE, SB], BF,
                                     isOutput=False)
    qwt_d = nc.declare_dram_parameter("qwt", [128, NE, DQ], BF, isOutput=False)
    qb_d = nc.declare_dram_parameter("qb", [DQ], F32, isOutput=False)
    kwt_d = nc.declare_dram_parameter("kwt", [128, NE, DKV], BF, isOutput=False)
    vwt_d = nc.declare_dram_parameter("vwt", [128, NE, DKV], BF, isOutput=False)
    owt_d = nc.declare_dram_parameter("owt", [128, NHL, E], BF, isOutput=False)
    # p-major output: one 512KB store per half row-block with 4KB contiguous
    # per-partition runs; host reassembles with a plain reshape
    out_d = nc.declare_dram_parameter("out", [NOB, 128, NOE, 512], BF,
                                      isOutput=True)

    Id = mybir.ActivationFunctionType.Identity
    Exp = mybir.ActivationFunctionType.Exp

    def copy_ps(i, dst, src):
        # alternate psum->sbuf copies between ACT and DVE
        if i % 2 == 0:
            nc.scalar.copy(dst, src)
        else:
            nc.vector.tensor_copy(dst, src)

    with tile.TileContext(nc) as tc, ExitStack() as es:
        consts = es.enter_context(tc.tile_pool(name="consts", bufs=1))
        persist = es.enter_context(tc.tile_pool(name="persist", bufs=1))

        # ---------------- constants ----------------
        ident = consts.tile([128, 128], BF)
        ones128 = consts.tile([128, 128], BF)
        nc.vector.memset(ones128[:, :], 1.0)

        qb_sb = consts.tile([128, NHL], F32)
        nc.sync.dma_start(qb_sb[:, :], qb_d[:].rearrange("(t p) -> p t", p=128))

        # warm the ACT exp table during the initial DMA wait
        warm = consts.tile([128, 1], F32)
        nc.scalar.activation(warm[:, :], qb_sb[:, 0:1], Exp, scale=1.0)

        # ---------------- persistent tensors ----------
        KT = persist.tile([128, S], BF)              # K^T [d, seq]
        QT = persist.tile([128, NHL, S], BF)         # Q^T per head [d, seq]
        Vn = persist.tile([128, NKT, 128], BF)       # V natural [seq, d] chunks
        owT = persist.tile([128, NHL, E], BF)        # o-proj weights

        # ================= phase 1: projections =================
        # group-burst interleaved stream: per e-group g the PE issues
        # bursts K(g),V(g),Q0(g)..Q3(g) into 6 live PSUM accumulators.
        # The first matmul needs only kwt-g0 + x-g0 (1.25MB), and each
        # stream's weights are needed one burst later than the previous
        # stream's, matching the slow early DMA ramp.
        xlast = persist.tile([128, NE, SB], BF)      # sb3 x, used in phase 2
        qwT = persist.tile([128, NE, DQ], BF)        # q weights, also phase 2

        with (
            tc.tile_pool(name="xz", bufs=2) as xz,
            tc.tile_pool(name="wproj", bufs=1) as wproj,
            tc.tile_pool(name="vtmp", bufs=2) as vtmp,
            tc.tile_pool(name="ps_p", bufs=1, space="PSUM") as ps_p,
            tc.tile_pool(name="ps_tr", bufs=2, space="PSUM") as ps_tr,
        ):
            kwT = wproj.tile([128, NE, DKV], BF, tag="kwT")
            vwT = wproj.tile([128, NE, DKV], BF, tag="vwT")
            xTb0 = xz.tile([128, NE, SB], BF, tag="x")
            probe = consts.tile([128, 1], BF)
            # urgent DMAs dispatched from parallel engine queues in NEED
            # order (transfers complete roughly in dispatch order)
            nc.gpsimd.dma_start(kwT[:, 0:8, :], kwt_d[:, 0:8, :])
            nc.scalar.dma_start(xTb0[:, 0:4, :], xt_d[0, :, 0:4, :])
            nc.gpsimd.dma_start(xTb0[:, 4:8, :], xt_d[0, :, 4:8, :])
            nc.gpsimd.dma_start(vwT[:, 0:8, :], vwt_d[:, 0:8, :])
            for g in range(1, 4):
                gs = slice(g * 8, (g + 1) * 8)
                nc.scalar.dma_start(xTb0[:, gs, :], xt_d[0, :, gs, :])
            # identity built after the two urgent gpsimd dispatches
            make_identity(nc, ident[:, :])
            for g in range(1, 4):
                gs = slice(g * 8, (g + 1) * 8)
                nc.gpsimd.dma_start(kwT[:, gs, :], kwt_d[:, gs, :])
                nc.gpsimd.dma_start(vwT[:, gs, :], vwt_d[:, gs, :])
            for g in range(4):
                gs = slice(g * 8, (g + 1) * 8)
                nc.sync.dma_start(qwT[:, gs, :], qwt_d[:, gs, :])
            # bulk fetches paced by data probes on the gpsimd queue: each
            # dispatch fires only once the previous transfer's tail landed,
            # keeping bulk traffic out of the urgent DMA window
            xblocks = [xTb0]
            for sb in range(1, NSB - 1):
                xn = xz.tile([128, NE, SB], BF, tag="x")
                xblocks.append(xn)
            xblocks.append(xlast)
            nc.gpsimd.tensor_copy(probe[:, :], qwT[:, 15, 127:128])
            nc.gpsimd.dma_start(xblocks[1][:, :, :], xt_d[1, :, :, :])
            nc.gpsimd.tensor_copy(probe[:, :], xblocks[1][:, 31, 511:512])
            nc.gpsimd.dma_start(xblocks[2][:, :, :], xt_d[2, :, :, :])
            nc.gpsimd.tensor_copy(probe[:, :], xblocks[2][:, 31, 511:512])
            nc.gpsimd.dma_start(xblocks[3][:, :, :], xt_d[3, :, :, :])
            nc.gpsimd.tensor_copy(probe[:, :], xblocks[3][:, 31, 511:512])
            nc.gpsimd.dma_start(owT[:, :, :], owt_d[:, :, :])

            # PE warm-up: one long accumulation group of dummy matmuls
            # (no per-MM semaphores) on the k accumulator's bank while the
            # first input DMAs land, so the HAM clock-gate is warm when
            # real matmuls start.  The K chain overwrites it (start=True).
            wt = ps_p.tile([128, SB], F32, tag="k")
            NWARM = 24
            for i in range(NWARM):
                nc.tensor.matmul(wt[:, 0:128], ident[:, :], ident[:, :],
                                 start=(i == 0), stop=(i == NWARM - 1))

            trans_pending = None
            for sb in range(NSB):
                xTb = xblocks[sb]
                ssl = slice(sb * SB, (sb + 1) * SB)
                # the last block's Q heads 0,1 are deferred into phase 2's
                # first q-block, whose slots are otherwise exp-latency-paced
                qheads = list(range(NHL)) if sb < NSB - 1 else [2, 3]
                trans_prev = trans_pending
                ps_qs = {}
                for h in qheads:
                    ps_q_h = ps_p.tile([128, SB], F32, tag=f"q{h}")
                    ps_qs[h] = ps_q_h
                ps_k = ps_p.tile([128, SB], F32, tag="k")
                ps_v = ps_p.tile([128, SB], F32, tag="v")
                for g in range(4):
                    ge = range(g * 8, (g + 1) * 8)
                    last = (g == 3)
                    for e in ge:
                        nc.tensor.matmul(ps_k[:, :], kwT[:, e, :],
                                         xTb[:, e, :],
                                         start=(e == 0), stop=(e == NE - 1))
                    if last:
                        nc.scalar.copy(KT[:, ssl], ps_k[:, :])
                    for e in ge:
                        nc.tensor.matmul(ps_v[:, :], vwT[:, e, :],
                                         xTb[:, e, :],
                                         start=(e == 0), stop=(e == NE - 1))
                    if last:
                        VTb = vtmp.tile([128, SB], BF, tag="vt")
                        nc.vector.tensor_copy(VTb[:, :], ps_v[:, :])
                        trans_pending = (VTb, sb)
                    # previous block's V transposes sprinkled between bursts
                    if trans_prev is not None:
                        VTprev, psb = trans_prev
                        tp = ps_tr.tile([128, 128], BF, tag="tr")
                        nc.tensor.transpose(
                            tp[:, :], VTprev[:, g * 128:(g + 1) * 128],
                            ident[:, :])
                        nc.vector.tensor_copy(Vn[:, psb * 4 + g, :], tp[:, :])
                    for n, h in enumerate(qheads):
                        for e in ge:
                            nc.tensor.matmul(
                                ps_qs[h][:, :],
                                qwT[:, e, h * 128:(h + 1) * 128],
                                xTb[:, e, :],
                                start=(e == 0), stop=(e == NE - 1))
                        if last:
                            if n % 2 == 0:
                                nc.scalar.activation(
                                    QT[:, h, ssl], ps_qs[h][:, :], Id,
                                    bias=qb_sb[:, h:h + 1])
                            else:
                                nc.vector.tensor_scalar_add(
                                    QT[:, h, ssl], ps_qs[h][:, :],
                                    qb_sb[:, h:h + 1])
            # last block's V transposes
            VTprev, psb = trans_pending
            for i in range(SB // 128):
                tp = ps_tr.tile([128, 128], BF, tag="tr")
                nc.tensor.transpose(tp[:, :], VTprev[:, i * 128:(i + 1) * 128],
                                    ident[:, :])
                nc.vector.tensor_copy(Vn[:, psb * 4 + i, :], tp[:, :])

        # ================= phase 2: attention + pipelined o-proj ==========
        with (
            tc.tile_pool(name="attn", bufs=2) as attn,
            tc.tile_pool(name="obp", bufs=4) as obp,
            tc.tile_pool(name="ps_s", bufs=3, space="PSUM") as ps_s,
            tc.tile_pool(name="ps_o", bufs=2, space="PSUM") as ps_o,
            tc.tile_pool(name="zpo", bufs=3, space="PSUM") as zpo,
        ):
            state = {"ob": None, "ci": 0}

            def emit_group(qprev, sl, ec, OTbSrc):
                # one o-proj group: out rows [qprev*4+sl], cols ec*512...
                trail = (qprev == NQS - 1)
                po = zpo.tile([128, 512], F32, tag="po")
                for dh in range(NHL):
                    nc.tensor.matmul(
                        po[:, :],
                        OTbSrc[:, dh, sl * 128:(sl + 1) * 128],
                        owT[:, dh, ec * 512:(ec + 1) * 512],
                        start=(dh == 0), stop=(dh == NHL - 1))
                blk = 4 * qprev + sl
                if trail and sl == NQS - 1:
                    # very last row block: column-pair stores so the tail
                    # drains quickly after the final matmul
                    if ec % 2 == 0:
                        ob = obp.tile([128, 2, 512], BF, tag="ob2")
                        state["ob"] = ob
                    copy_ps(state["ci"], state["ob"][:, ec % 2, :], po[:, :])
                    if ec % 2 == 1:
                        e0 = (ec // 2) * 2
                        nc.scalar.dma_start(out_d[blk, :, e0:e0 + 2, :],
                                            state["ob"][:, :, :])
                else:
                    if ec % 4 == 0:
                        ob = obp.tile([128, 4, 512], BF, tag="ob")
                        state["ob"] = ob
                    copy_ps(state["ci"], state["ob"][:, ec % 4, :], po[:, :])
                    if ec % 4 == 3:
                        e0 = (ec // 4) * 4
                        nc.sync.dma_start(out_d[blk, :, e0:e0 + 4, :],
                                          state["ob"][:, :, :])
                state["ci"] += 1

            groups = []          # pending o-proj groups (qprev, sl, ec, src)
            pending_z = None     # (acc, bcs target slot) awaiting Z matmul
            # Q(sb3) heads 0,1 deferred from phase 1: two chain matmuls per
            # q-block-0 slot fill the exp-latency pacing gaps there
            dq = [(h, e) for h in (0, 1) for e in range(NE)]
            dq_state = {}

            def emit_dq():
                h, e = dq.pop(0)
                if e == 0:
                    qa = zpo.tile([128, SB], F32, tag="po")
                    dq_state["qa"] = qa
                nc.tensor.matmul(
                    dq_state["qa"][:, :],
                    qwT[:, e, h * 128:(h + 1) * 128],
                    xlast[:, e, :],
                    start=(e == 0), stop=(e == NE - 1))
                if e == NE - 1:
                    ssl3 = slice(3 * SB, 4 * SB)
                    if h == 0:
                        nc.scalar.activation(
                            QT[:, h, ssl3], dq_state["qa"][:, :], Id,
                            bias=qb_sb[:, h:h + 1])
                    else:
                        nc.vector.tensor_scalar_add(
                            QT[:, h, ssl3], dq_state["qa"][:, :],
                            qb_sb[:, h:h + 1])

            def emit_z(pz):
                accz, bcsz = pz
                zps = zpo.tile([128, QS], F32, tag="po")
                nc.tensor.matmul(zps[:, :], ones128[:, :], accz[:, :],
                                 start=True, stop=True)
                nc.vector.reciprocal_approx_fast(bcsz[:, :], zps[:, :])

            OTbPrev = None
            for qi in range(NQS):
                qsl = slice(qi * QS, (qi + 1) * QS)
                OTb = attn.tile([128, NHL, QS], BF, tag="OTb")
                if OTbPrev is not None:
                    groups.extend(
                        (qi - 1, g // NOE, g % NOE, OTbPrev)
                        for g in range(NQS * NOE))
                PTprev = None
                ops_prev = None
                bcs_prev = None
                for h in range(NHL):
                    PT = attn.tile([128, NKT, QS], BF, tag="PT")
                    acc = attn.tile([128, QS], BF, tag="acc")
                    for kp in range(NKT // 2):
                        sps_pair = []
                        for j in range(2):
                            kt = kp * 2 + j
                            sps = ps_s.tile([128, QS], F32, tag="s")
                            sps_pair.append(sps)
                            nc.tensor.matmul(sps[:, :],
                                             KT[:, kt * 128:(kt + 1) * 128],
                                             QT[:, h, qsl],
                                             start=True, stop=True)
                        for j in range(2):
                            nc.scalar.activation(PT[:, kp * 2 + j, :],
                                                 sps_pair[j][:, :], Exp,
                                                 scale=EXP_SCALE)
                        # Z partial sums trail exp by one slot
                        if kp == 0:
                            nc.vector.tensor_add(acc[:, :], PT[:, 0, :],
                                                 PT[:, 1, :])
                        else:
                            nc.vector.tensor_add(acc[:, :], acc[:, :],
                                                 PT[:, kp * 2, :])
                            nc.vector.tensor_add(acc[:, :], acc[:, :],
                                                 PT[:, kp * 2 + 1, :])
                        # PV of the previous head fills exp-latency gaps
                        if h >= 1:
                            for j in range(2):
                                kt = kp * 2 + j
                                nc.tensor.matmul(ops_prev[:, :], Vn[:, kt, :],
                                                 PTprev[:, kt, :],
                                                 start=(kt == 0),
                                                 stop=(kt == NKT - 1))
                        # one pending o-proj group per slot (shifted so a
                        # group never chases a just-produced OTb)
                        if groups and not (h == 0 and kp == 0):
                            emit_group(*groups.pop(0))
                        # the previous head's Z matmul, one slot into this
                        # row so the DVE chunk-add chain has settled
                        if kp == 0 and pending_z is not None:
                            emit_z(pending_z)
                            pending_z = None
                        if dq and qi == 0:
                            emit_dq()
                            emit_dq()
                    # end of head row
                    bcs = attn.tile([128, QS], F32, tag="bcs")
                    pending_z = (acc, bcs)
                    if h >= 1:
                        nc.vector.tensor_mul(OTb[:, h - 1, :], ops_prev[:, :],
                                             bcs_prev[:, :])
                    ops_cur = ps_o.tile([128, QS], F32, tag="o")
                    PTprev, ops_prev, bcs_prev = PT, ops_cur, bcs
                # trailing PV + Z + normalize for the last head of this block
                for kt in range(NKT):
                    nc.tensor.matmul(ops_prev[:, :], Vn[:, kt, :],
                                     PTprev[:, kt, :],
                                     start=(kt == 0), stop=(kt == NKT - 1))
                    if kt == 1 and pending_z is not None:
                        emit_z(pending_z)
                        pending_z = None
                    if kt >= 2 and kt % 2 == 0 and groups:
                        emit_group(*groups.pop(0))
                nc.vector.tensor_mul(OTb[:, NHL - 1, :], ops_prev[:, :],
                                     bcs_prev[:, :])
                OTbPrev = OTb
            # trailing o-proj for the last q-block
            groups.extend((NQS - 1, g // NOE, g % NOE, OTbPrev)
                          for g in range(NQS * NOE))
            while groups:
                emit_group(*groups.pop(0))

    nc.finalize()
    return nc


def make_in_maps(x, q_w, q_b, k_w, k_b, v_w, v_b, o_w):
    x2 = np.asarray(x, np.float32).reshape(S, E)
    # xt[sb, p, g, sl] = x[sb*SB+sl, g*128+p]
    xt = np.ascontiguousarray(
        x2.T.reshape(NE, 128, NSB, SB).transpose(2, 1, 0, 3)).astype(BF_NP)
    q_w = np.asarray(q_w, np.float32)
    k_w = np.asarray(k_w, np.float32)
    v_w = np.asarray(v_w, np.float32)
    o_w = np.asarray(o_w, np.float32)
    in_maps = []
    for c in range(NCORES):
        qsl = slice(c * DQ, (c + 1) * DQ)
        ksl = slice(c * DKV, (c + 1) * DKV)
        # w^T [E, dout] chunked to [p, g, dout]
        qwt = q_w[qsl].T.reshape(NE, 128, DQ).transpose(1, 0, 2)
        kwt = k_w[ksl].T.reshape(NE, 128, DKV).transpose(1, 0, 2)
        vwt = v_w[ksl].T.reshape(NE, 128, DKV).transpose(1, 0, 2)
        # o_w slice^T [DQ, E] chunked to [p, h, E]
        owt = o_w[:, qsl].T.reshape(NHL, 128, E).transpose(1, 0, 2)
        in_maps.append({
            "xt": xt,
            "qwt": np.ascontiguousarray(qwt).astype(BF_NP),
            "qb": np.ascontiguousarray(np.asarray(q_b, np.float32)[qsl]),
            "kwt": np.ascontiguousarray(kwt).astype(BF_NP),
            "vwt": np.ascontiguousarray(vwt).astype(BF_NP),
            "owt": np.ascontiguousarray(owt).astype(BF_NP),
        })
    return in_maps


def kernel(x, q_w, q_b, k_w, k_b, v_w, v_b, o_w, o_b):
    global _CACHED_NC
    in_maps = make_in_maps(x, q_w, q_b, k_w, k_b, v_w, v_b, o_w)
    if _CACHED_NC is None:
        _CACHED_NC = build_bass()
    res = run_bass_kernel_spmd(_CACHED_NC, in_maps, list(range(NCORES)))
    out = np.zeros((S, E), np.float64)
    for i in range(NCORES):
        out += res.results[i]["out"].astype(np.float32).reshape(S, E)
    # host-folded biases: o_b plus the V-bias term (sum_k P/Z = 1 makes the
    # V bias contribute v_bias_full @ o_w.T, constant over positions); k_b
    # cancels in softmax and is dropped entirely.
    v_bias_full = np.repeat(
        np.asarray(v_b, np.float64).reshape(NCORES, D), NHL, axis=0).reshape(E)
    out += np.asarray(o_b, np.float64)
    out += v_bias_full @ np.asarray(o_w, np.float64).T
    return out.astype(np.float32).reshape(1, S, E)


# revision 37
# speedup vs baseline: 1.2284x; 1.2284x over previous
"""GQA (softermax) Trainium2 kernel, tensor-parallel over kv-head groups.

Problem: x[1,2048,4096], 32 q-heads / 8 kv-heads, head_dim=128, base-2
softmax (softermax), fp32 reference. Each of the 8 cores owns one kv-head
group (4 q-heads, 512 q dims, 128 kv dims) and computes a partial
o-projection [2048,4096]; the host sums the 8 partials and adds o_b.

v5 (pipelined, all-bf16; 458us -> ~421us). fp8/DoubleRow was evaluated
and rejected: softmax rows here are peaked (N_eff down to ~40), so fp8
quantization of P or x costs 2.2e-2..4.8e-2 max-rel error vs the 2e-2
gate. All matmuls stay bf16; the wins are scheduling:
 - k_b dropped entirely (adds a per-q constant to scores -> cancels in
   softmax); v_b folded into a host-side constant (sum_k P/Z = 1, so the
   V-bias contribution to out is v_bias_full @ o_w.T, independent of s).
 - PE warm-up: dummy matmuls during the initial DMA wait keep the HAM
   clock-gate warm so real matmuls start at 2.4 GHz.
 - Phase 1 is a group-burst interleaved stream (per e-group: K,V,Q0..Q3
   bursts accumulating into 6 live PSUM banks) so the weight/x DMA
   need-curve stays under the slow early DMA ramp and the first matmul
   needs only 1.25MB; urgent DMAs dispatch in parallel from
   scalar/gpsimd/sync queues in need order; bulk x-blocks are paced by
   data-probe reads on the gpsimd queue; PSUM->SBUF copies run inline
   after each stream's last burst, alternating ACT/DVE.
 - Phase-2 software pipeline: per (head h, chunk-pair kp) slot emits
   2 score MMs + exp + 2 PV MMs of head h-1 + one o-proj group (sl,ec)
   of the previous q-block, so the PE never waits on the ACT exp and
   the o-projection fills all exp-latency gaps. Z chunk-adds (DVE) are
   emitted per-slot; the Z broadcast matmul is deferred one slot.
 - Q(sb3) heads 0,1 are deferred from phase 1 into q-block-0's slots
   (which have no o-proj work and would otherwise be exp-paced).
 - o-proj output DMA'd per half-row-block (512KB); the final row block
   stores column-pairs on the scalar queue to drain the tail.

Per-core dataflow:
  proj:  xT[e,s] (DMA, host-chunked) ; K^T,V^T,Q^T = W^T.T @ xT
         V natural via 16 PE transposes of V^T
  attn:  S^T[k,q] = KT_chunk.T @ QT (pairs of chunks into one 2-bank
         PSUM tile); P^T = exp(S^T * ln2/sqrt(128)) [ACT, FD=1024]
         O^T[d,q] = sum_k Vn_chunk.T @ P^T_chunk   (PSUM accum)
         acc = bf16 chunk-add chain of P^T [DVE]; Z = ones128.T @ acc
         OTb = O^T * recip_approx(Z)               [DVE]
  oproj: out[s,e] = sum_h OTb_h_chunk.T @ owT_h    (partial; host sums)
"""

import math
from contextlib import ExitStack

import numpy as np
import ml_dtypes

import concourse.bass as bass
from concourse import bacc
import concourse.mybir as mybir
import concourse.tile as tile
from concourse.bass_utils import run_bass_kernel_spmd
from concourse.masks import make_identity

E = 4096          # embed dim
S = 2048          # sequence
D = 128           # head dim
NHL = 4           # q heads per core
DQ = NHL * D      # 512 q dims per core
DKV = 128         # kv dims per core (1 kv head)
NCORES = 8
NE = E // 128     # 32 embed chunks

SB = 512          # seq block for projection pass
NSB = S // SB
QS = 512          # q block in attention
NQS = S // QS
NKT = S // 128    # 16 k chunks
NOB = S // 128    # 16 output row blocks
NOE = E // 512    # 8 output col blocks

F32 = mybir.dt.float32
BF = mybir.dt.bfloat16
BF_NP = ml_dtypes.bfloat16
EXP_SCALE = math.log(2.0) / math.sqrt(D)

_CACHED_NC = None


def build_bass():
    nc = bacc.Bacc(None)

    # all inputs pre-chunked on host into SBUF tile layout [p, chunk, cols]
    xt_d = nc.declare_dram_parameter("xt", [NSB, 128, NE, SB], BF,
                                     isOutput=False)
    qwt_d = nc.declare_dram_parameter("qwt", [128, NE, DQ], BF, isOutput=False)
    qb_d = nc.declare_dram_parameter("qb", [DQ], F32, isOutput=False)
    kwt_d = nc.declare_dram_parameter("kwt", [128, NE, DKV], BF, isOutput=False)
    vwt_d = nc.declare_dram_parameter("vwt", [128, NE, DKV], BF, isOutput=False)
    owt_d = nc.declare_dram_parameter("owt", [128, NHL, E], BF, isOutput=False)
    # p-major output: one 512KB store per half row-block with 4KB contiguous
    # per-partition runs; host reassembles with a plain reshape
    out_d = nc.declare_dram_parameter("out", [NOB, 128, NOE, 512], BF,
                                      isOutput=True)

    Id = mybir.ActivationFunctionType.Identity
    Exp = mybir.ActivationFunctionType.Exp

    def copy_ps(i, dst, src):
        # alternate psum->sbuf copies between ACT and DVE
        if i % 2 == 0:
            nc.scalar.copy(dst, src)
        else:
            nc.vector.tensor_copy(dst, src)

    with tile.TileContext(nc) as tc, ExitStack() as es:
        consts = es.enter_context(tc.tile_pool(name="consts", bufs=1))
        persist = es.enter_context(tc.tile_pool(name="persist", bufs=1))

        # ---------------- constants ----------------
        ident = consts.tile([128, 128], BF)
        ones128 = consts.tile([128, 128], BF)
        nc.vector.memset(ones128[:, :], 1.0)

        qb_sb = consts.tile([128, NHL], F32)
        nc.sync.dma_start(qb_sb[:, :], qb_d[:].rearrange("(t p) -> p t", p=128))

        # warm the ACT exp table during the initial DMA wait
        warm = consts.tile([128, 1], F32)
        nc.scalar.activation(warm[:, :], qb_sb[:, 0:1], Exp, scale=1.0)

        # ---------------- persistent tensors ----------
        KT = persist.tile([128, S], BF)              # K^T [d, seq]
        QT = persist.tile([128, NHL, S], BF)         # Q^T per head [d, seq]
        Vn = persist.tile([128, NKT, 128], BF)       # V natural [seq, d] chunks
        owT = persist.tile([128, NHL, E], BF)        # o-proj weights

        # ================= phase 1: projections =================
        # group-burst interleaved stream: per e-group g the PE issues
        # bursts K(g),V(g),Q0(g)..Q3(g) into 6 live PSUM accumulators.
        # The first matmul needs only kwt-g0 + x-g0 (1.25MB), and each
        # stream's weights are needed one burst later than the previous
        # stream's, matching the slow early DMA ramp.
        xlast = persist.tile([128, NE, SB], BF)      # sb3 x, used in phase 2
        qwT = persist.tile([128, NE, DQ], BF)        # q weights, also phase 2

        with (
            tc.tile_pool(name="xz", bufs=2) as xz,
            tc.tile_pool(name="wproj", bufs=1) as wproj,
            tc.tile_pool(name="vtmp", bufs=2) as vtmp,
            tc.tile_pool(name="ps_p", bufs=1, space="PSUM") as ps_p,
            tc.tile_pool(name="ps_tr", bufs=2, space="PSUM") as ps_tr,
        ):
            kwT = wproj.tile([128, NE, DKV], BF, tag="kwT")
            vwT = wproj.tile([128, NE, DKV], BF, tag="vwT")
            xTb0 = xz.tile([128, NE, SB], BF, tag="x")
            probe = consts.tile([128, 1], BF)
            # urgent DMAs dispatched from parallel engine queues in NEED
            # order (transfers complete roughly in dispatch order)
            nc.gpsimd.dma_start(kwT[:, 0:8, :], kwt_d[:, 0:8, :])
            nc.gpsimd.dma_start(vwT[:, 0:8, :], vwt_d[:, 0:8, :])
            for g in range(4):
                gs = slice(g * 8, (g + 1) * 8)
                nc.scalar.dma_start(xTb0[:, gs, :], xt_d[0, :, gs, :])
            # identity built after the two urgent gpsimd dispatches
            make_identity(nc, ident[:, :])
            for g in range(1, 4):
                gs = slice(g * 8, (g + 1) * 8)
                nc.gpsimd.dma_start(kwT[:, gs, :], kwt_d[:, gs, :])
                nc.gpsimd.dma_start(vwT[:, gs, :], vwt_d[:, gs, :])
            for g in range(4):
                gs = slice(g * 8, (g + 1) * 8)
                nc.sync.dma_start(qwT[:, gs, :], qwt_d[:, gs, :])
            # bulk fetches paced by data probes on the gpsimd queue: each
            # dispatch fires only once the previous transfer's tail landed,
            # keeping bulk traffic out of the urgent DMA window
            xblocks = [xTb0]
            for sb in range(1, NSB - 1):
                xn = xz.tile([128, NE, SB], BF, tag="x")
                xblocks.append(xn)
            xblocks.append(xlast)
            nc.gpsimd.tensor_copy(probe[:, :], qwT[:, 15, 127:128])
            nc.gpsimd.dma_start(xblocks[1][:, :, :], xt_d[1, :, :, :])
            nc.gpsimd.tensor_copy(probe[:, :], xblocks[1][:, 31, 511:512])
            nc.gpsimd.dma_start(xblocks[2][:, :, :], xt_d[2, :, :, :])
            nc.gpsimd.tensor_copy(probe[:, :], xblocks[2][:, 31, 511:512])
            nc.gpsimd.dma_start(xblocks[3][:, :, :], xt_d[3, :, :, :])
            nc.gpsimd.tensor_copy(probe[:, :], xblocks[3][:, 31, 511:512])
            nc.gpsimd.dma_start(owT[:, :, :], owt_d[:, :, :])

            # PE warm-up: one long accumulation group of dummy matmuls
            # (no per-MM semaphores) on the k accumulator's bank while the
            # first input DMAs land, so the HAM clock-gate is warm when
            # real matmuls start.  The K chain overwrites it (start=True).
            wt = ps_p.tile([128, SB], F32, tag="k")
            NWARM = 28
            for i in range(NWARM):
                nc.tensor.matmul(wt[:, 0:128], ident[:, :], ident[:, :],
                                 start=(i == 0), stop=(i == NWARM - 1))

            trans_pending = None
            for sb in range(NSB):
                xTb = xblocks[sb]
                ssl = slice(sb * SB, (sb + 1) * SB)
                # the last block's Q heads 0,1 are deferred into phase 2's
                # first q-block, whose slots are otherwise exp-latency-paced
                qheads = list(range(NHL)) if sb < NSB - 1 else [2, 3]
                trans_prev = trans_pending
                ps_qs = {}
                for h in qheads:
                    ps_q_h = ps_p.tile([128, SB], F32, tag=f"q{h}")
                    ps_qs[h] = ps_q_h
                ps_k = ps_p.tile([128, SB], F32, tag="k")
                ps_v = ps_p.tile([128, SB], F32, tag="v")
                for g in range(4):
                    ge = range(g * 8, (g + 1) * 8)
                    last = (g == 3)
                    for e in ge:
                        nc.tensor.matmul(ps_k[:, :], kwT[:, e, :],
                                         xTb[:, e, :],
                                         start=(e == 0), stop=(e == NE - 1))
                    if last:
                        nc.scalar.copy(KT[:, ssl], ps_k[:, :])
                    for e in ge:
                        nc.tensor.matmul(ps_v[:, :], vwT[:, e, :],
                                         xTb[:, e, :],
                                         start=(e == 0), stop=(e == NE - 1))
                    if last:
                        VTb = vtmp.tile([128, SB], BF, tag="vt")
                        nc.vector.tensor_copy(VTb[:, :], ps_v[:, :])
                        trans_pending = (VTb, sb)
                    # previous block's V transposes sprinkled between bursts
                    if trans_prev is not None:
                        VTprev, psb = trans_prev
                        tp = ps_tr.tile([128, 128], BF, tag="tr")
                        nc.tensor.transpose(
                            tp[:, :], VTprev[:, g * 128:(g + 1) * 128],
                            ident[:, :])
                        nc.vector.tensor_copy(Vn[:, psb * 4 + g, :], tp[:, :])
                    for n, h in enumerate(qheads):
                        for e in ge:
                            nc.tensor.matmul(
                                ps_qs[h][:, :],
                                qwT[:, e, h * 128:(h + 1) * 128],
                                xTb[:, e, :],
                                start=(e == 0), stop=(e == NE - 1))
                        if last:
                            if n % 2 == 0:
                                nc.scalar.activation(
                                    QT[:, h, ssl], ps_qs[h][:, :], Id,
                                    bias=qb_sb[:, h:h + 1])
                            else:
                                nc.vector.tensor_scalar_add(
                                    QT[:, h, ssl], ps_qs[h][:, :],
                                    qb_sb[:, h:h + 1])
            # last block's V transposes
            VTprev, psb = trans_pending
            for i in range(SB // 128):
                tp = ps_tr.tile([128, 128], BF, tag="tr")
                nc.tensor.transpose(tp[:, :], VTprev[:, i * 128:(i + 1) * 128],
                                    ident[:, :])
                nc.vector.tensor_copy(Vn[:, psb * 4 + i, :], tp[:, :])

        # ================= phase 2: attention + pipelined o-proj ==========
        with (
            tc.tile_pool(name="attn", bufs=2) as attn,
            tc.tile_pool(name="obp", bufs=4) as obp,
            tc.tile_pool(name="ps_s", bufs=3, space="PSUM") as ps_s,
            tc.tile_pool(name="ps_o", bufs=2, space="PSUM") as ps_o,
            tc.tile_pool(name="zpo", bufs=3, space="PSUM") as zpo,
        ):
            state = {"ob": None, "ci": 0}

            def emit_group(qprev, sl, ec, OTbSrc):
                # one o-proj group: out rows [qprev*4+sl], cols ec*512...
                trail = (qprev == NQS - 1)
                po = zpo.tile([128, 512], F32, tag="po")
                for dh in range(NHL):
                    nc.tensor.matmul(
                        po[:, :],
                        OTbSrc[:, dh, sl * 128:(sl + 1) * 128],
                        owT[:, dh, ec * 512:(ec + 1) * 512],
                        start=(dh == 0), stop=(dh == NHL - 1))
                blk = 4 * qprev + sl
                if trail and sl == NQS - 1:
                    # very last row block: column-pair stores so the tail
                    # drains quickly after the final matmul
                    if ec % 2 == 0:
                        ob = obp.tile([128, 2, 512], BF, tag="ob2")
                        state["ob"] = ob
                    copy_ps(state["ci"], state["ob"][:, ec % 2, :], po[:, :])
                    if ec % 2 == 1:
                        e0 = (ec // 2) * 2
                        nc.scalar.dma_start(out_d[blk, :, e0:e0 + 2, :],
                                            state["ob"][:, :, :])
                else:
                    if ec % 4 == 0:
                        ob = obp.tile([128, 4, 512], BF, tag="ob")
                        state["ob"] = ob
                    copy_ps(state["ci"], state["ob"][:, ec % 4, :], po[:, :])
                    if ec % 4 == 3:
                        e0 = (ec // 4) * 4
                        nc.sync.dma_start(out_d[blk, :, e0:e0 + 4, :],
                                          state["ob"][:, :, :])
                state["ci"] += 1

            groups = []          # pending o-proj groups (qprev, sl, ec, src)
            pending_z = None     # (acc, bcs target slot) awaiting Z matmul
            # Q(sb3) heads 0,1 deferred from phase 1: two chain matmuls per
            # q-block-0 slot fill the exp-latency pacing gaps there
            dq = [(h, e) for h in (0, 1) for e in range(NE)]
            dq_state = {}

            def emit_dq():
                h, e = dq.pop(0)
                if e == 0:
                    qa = zpo.tile([128, SB], F32, tag="po")
                    dq_state["qa"] = qa
                nc.tensor.matmul(
                    dq_state["qa"][:, :],
                    qwT[:, e, h * 128:(h + 1) * 128],
                    xlast[:, e, :],
                    start=(e == 0), stop=(e == NE - 1))
                if e == NE - 1:
                    ssl3 = slice(3 * SB, 4 * SB)
                    if h == 0:
                        nc.scalar.activation(
                            QT[:, h, ssl3], dq_state["qa"][:, :], Id,
                            bias=qb_sb[:, h:h + 1])
                    else:
                        nc.vector.tensor_scalar_add(
                            QT[:, h, ssl3], dq_state["qa"][:, :],
                            qb_sb[:, h:h + 1])

            def emit_z(pz):
                accz, bcsz = pz
                zps = zpo.tile([128, QS], F32, tag="po")
                nc.tensor.matmul(zps[:, :], ones128[:, :], accz[:, :],
                                 start=True, stop=True)
                nc.vector.reciprocal_approx_fast(bcsz[:, :], zps[:, :])

            OTbPrev = None
            for qi in range(NQS):
                qsl = slice(qi * QS, (qi + 1) * QS)
                OTb = attn.tile([128, NHL, QS], BF, tag="OTb")
                if OTbPrev is not None:
                    groups.extend(
                        (qi - 1, g // NOE, g % NOE, OTbPrev)
                        for g in range(NQS * NOE))
                PTprev = None
                ops_prev = None
                bcs_prev = None
                for h in range(NHL):
                    PT = attn.tile([128, NKT, QS], BF, tag="PT")
                    acc = attn.tile([128, QS], BF, tag="acc")
                    for kp in range(NKT // 2):
                        sps_pair = []
                        for j in range(2):
                            kt = kp * 2 + j
                            sps = ps_s.tile([128, QS], F32, tag="s")
                            sps_pair.append(sps)
                            nc.tensor.matmul(sps[:, :],
                                             KT[:, kt * 128:(kt + 1) * 128],
                                             QT[:, h, qsl],
                                             start=True, stop=True)
                        for j in range(2):
                            nc.scalar.activation(PT[:, kp * 2 + j, :],
                                                 sps_pair[j][:, :], Exp,
                                                 scale=EXP_SCALE)
                        # Z partial sums trail exp by one slot
                        if kp == 0:
                            nc.vector.tensor_add(acc[:, :], PT[:, 0, :],
                                                 PT[:, 1, :])
                        else:
                            nc.vector.tensor_add(acc[:, :], acc[:, :],
                                                 PT[:, kp * 2, :])
                            nc.vector.tensor_add(acc[:, :], acc[:, :],
                                                 PT[:, kp * 2 + 1, :])
                        # PV of the previous head fills exp-latency gaps
                        if h >= 1:
                            for j in range(2):
                                kt = kp * 2 + j
                                nc.tensor.matmul(ops_prev[:, :], Vn[:, kt, :],
                                                 PTprev[:, kt, :],
                                                 start=(kt == 0),
                                                 stop=(kt == NKT - 1))
                        # one pending o-proj group per slot (shifted so a
                        # group never chases a just-produced OTb)
                        if groups and not (h == 0 and kp == 0):
                            emit_group(*groups.pop(0))
                        # the previous head's Z matmul, one slot into this
                        # row so the DVE chunk-add chain has settled
                        if kp == 0 and pending_z is not None:
                            emit_z(pending_z)
                            pending_z = None
                        if dq and qi == 0:
                            emit_dq()
                            emit_dq()
                    # end of head row
                    bcs = attn.tile([128, QS], F32, tag="bcs")
                    pending_z = (acc, bcs)
                    if h >= 1:
                        nc.vector.tensor_mul(OTb[:, h - 1, :], ops_prev[:, :],
                                             bcs_prev[:, :])
                    ops_cur = ps_o.tile([128, QS], F32, tag="o")
                    PTprev, ops_prev, bcs_prev = PT, ops_cur, bcs
                # trailing PV + Z + normalize for the last head of this block
                for kt in range(NKT):
                    nc.tensor.matmul(ops_prev[:, :], Vn[:, kt, :],
                                     PTprev[:, kt, :],
                                     start=(kt == 0), stop=(kt == NKT - 1))
                    if kt == 1 and pending_z is not None:
                        emit_z(pending_z)
                        pending_z = None
                    if kt >= 2 and kt % 2 == 0 and groups:
                        emit_group(*groups.pop(0))
                nc.vector.tensor_mul(OTb[:, NHL - 1, :], ops_prev[:, :],
                                     bcs_prev[:, :])
                OTbPrev = OTb
            # trailing o-proj for the last q-block
            groups.extend((NQS - 1, g // NOE, g % NOE, OTbPrev)
                          for g in range(NQS * NOE))
            while groups:
                emit_group(*groups.pop(0))

    nc.finalize()
    return nc


def make_in_maps(x, q_w, q_b, k_w, k_b, v_w, v_b, o_w):
    x2 = np.asarray(x, np.float32).reshape(S, E)
    # xt[sb, p, g, sl] = x[sb*SB+sl, g*128+p]
    xt = np.ascontiguousarray(
        x2.T.reshape(NE, 128, NSB, SB).transpose(2, 1, 0, 3)).astype(BF_NP)
    q_w = np.asarray(q_w, np.float32)
    k_w = np.asarray(k_w, np.float32)
    v_w = np.asarray(v_w, np.float32)
    o_w = np.asarray(o_w, np.float32)
    in_maps = []
    for c in range(NCORES):
        qsl = slice(c * DQ, (c + 1) * DQ)
        ksl = slice(c * DKV, (c + 1) * DKV)
        # w^T [E, dout] chunked to [p, g, dout]
        qwt = q_w[qsl].T.reshape(NE, 128, DQ).transpose(1, 0, 2)
        kwt = k_w[ksl].T.reshape(NE, 128, DKV).transpose(1, 0, 2)
        vwt = v_w[ksl].T.reshape(NE, 128, DKV).transpose(1, 0, 2)
        # o_w slice^T [DQ, E] chunked to [p, h, E]
        owt = o_w[:, qsl].T.reshape(NHL, 128, E).transpose(1, 0, 2)
        in_maps.append({
            "xt": xt,
            "qwt": np.ascontiguousarray(qwt).astype(BF_NP),
            "qb": np.ascontiguousarray(np.asarray(q_b, np.float32)[qsl]),
            "kwt": np.ascontiguousarray(kwt).astype(BF_NP),
            "vwt": np.ascontiguousarray(vwt).astype(BF_NP),
            "owt": np.ascontiguousarray(owt).astype(BF_NP),
        })
    return in_maps


def kernel(x, q_w, q_b, k_w, k_b, v_w, v_b, o_w, o_b):
    global _CACHED_NC
    in_maps = make_in_maps(x, q_w, q_b, k_w, k_b, v_w, v_b, o_w)
    if _CACHED_NC is None:
        _CACHED_NC = build_bass()
    res = run_bass_kernel_spmd(_CACHED_NC, in_maps, list(range(NCORES)))
    out = np.zeros((S, E), np.float64)
    for i in range(NCORES):
        out += res.results[i]["out"].astype(np.float32).reshape(S, E)
    # host-folded biases: o_b plus the V-bias term (sum_k P/Z = 1 makes the
    # V bias contribute v_bias_full @ o_w.T, constant over positions); k_b
    # cancels in softmax and is dropped entirely.
    v_bias_full = np.repeat(
        np.asarray(v_b, np.float64).reshape(NCORES, D), NHL, axis=0).reshape(E)
    out += np.asarray(o_b, np.float64)
    out += v_bias_full @ np.asarray(o_w, np.float64).T
    return out.astype(np.float32).reshape(1, S, E)
